# revision 1
# baseline (speedup 1.0000x reference)
"""Trainium2 Bass kernel for nn_LogBezierButtress.

Math (per point n, per permutation p of the 8 input dims):
  B[d,q]  = C(19,q) x_d^q (1-x_d)^(19-q)          (Bernstein basis, O=20)
  mean chain:  f_0 = exp(meanw0[p]) * B[perm[p,0]]
               f_i = (f_{i-1} @ exp(meanw[i-1,p])) * B[perm[p,i]]
  var chains k=1..6 use weights exp(2*meanw + k*varw) and gate B^2.
  mean(n) = sum_{p,q} f_7 ; var(n) = sum_k c_k sum_{p,q} acc_7[k]

Device mapping (per core, points sharded 8 ways):
  - states live as [120, FD] fp16 SBUF tiles: 6 chains x 20 basis rows,
    points on the free dim. 24 packs: 20 "var" packs (k=1..6 of one p) and
    4 "mean" packs (mean chains of 6 p's).
  - per step: block-diag [120,120] fp16 matmul into PSUM, then gate
    multiply by a basis "stack" tile. Gate stacks are built by SBUF->SBUF
    DMA block copies from per-dim base tiles B / B^2, which are produced
    on device: Ln(x), Ln(1-x) -> selector matmuls (q*lnx+(19-q)*ln1x)
    -> ACT Exp with per-partition log-binomial bias.
  - the k-coefficients c_k are folded into the last var weights; the final
    sum over (chains, q) is a ones-vector matmul accumulated over packs.
"""

import sys

sys.path.insert(0, "/opt/trn_rl_repo")

from contextlib import ExitStack
from math import comb

import numpy as np

import concourse.bacc as bacc
import concourse.mybir as mybir
import concourse.tile as tile
from concourse.bass_utils import run_bass_kernel_spmd

N, D, ORDER, P = 32768, 8, 19, 20
O = ORDER + 1
NCORES = 8
NPC = N // NCORES  # points per core
FD = 1024          # points per tile (free dim)
NPACK = 24
C_COEF = [1.0, 1 / 2, 1 / 6, 1 / 24, 1 / 120, 1 / 720]
GROUPS = [list(range(g * 6, min(P, g * 6 + 6))) for g in range(4)]
# pack list: ("var", p) x20 then ("mean", group) x4
PACKS = [("var", p) for p in range(P)] + [("mean", g) for g in range(4)]
# packs whose gate runs fused on DVE (PSUM*SBUF->SBUF); the rest use an
# ACT copy (PSUM->SBUF fp16) + DVE fp16 2x multiply. Balance ACT vs DVE.
FUSED_PACKS = set(range(7))

f32 = mybir.dt.float32
f16 = mybir.dt.float16
AF = mybir.ActivationFunctionType


def _prep_consts(perm, meanw0, meanw, varw0, varw):
    """Host-side weight packing (small, O(P*O^2*D))."""
    perm = np.asarray(perm)
    m0 = np.asarray(meanw0, np.float64)
    mw = np.asarray(meanw, np.float64)
    v0 = np.asarray(varw0, np.float64)
    vw = np.asarray(varw, np.float64)

    wlhs = np.zeros((120, D - 1, NPACK, 120), np.float32)
    for i in range(1, D):
        for pk, (kind, val) in enumerate(PACKS):
            for b in range(6):
                if kind == "var":
                    W = np.exp(2 * mw[i - 1, val] + (b + 1) * vw[i - 1, val])
                    if i == D - 1:
                        W = W * C_COEF[b]
                else:
                    mem = GROUPS[val]
                    W = np.exp(mw[i - 1, mem[b]]) if b < len(mem) else np.zeros((O, O))
                wlhs[20 * b : 20 * b + 20, i - 1, pk, 20 * b : 20 * b + 20] = W

    w0v = np.zeros((120, NPACK), np.float32)
    onesr = np.zeros((120, NPACK, 2), np.float32)
    for pk, (kind, val) in enumerate(PACKS):
        for b in range(6):
            sl = slice(20 * b, 20 * b + 20)
            if kind == "var":
                w0v[sl, pk] = np.exp(2 * m0[val, 0] + (b + 1) * v0[val, 0])
                onesr[sl, pk, 1] = 1.0
            else:
                mem = GROUPS[val]
                if b < len(mem):
                    w0v[sl, pk] = np.exp(m0[mem[b], 0])
                    onesr[sl, pk, 0] = 1.0

    # selector matmul weights: Z[(d%4)*20+q, n] = q*lnx[d,n] + (19-q)*ln1x[d,n]
    sel = np.zeros((8, 4, 80), np.float32)
    for h in range(2):
        for dd in range(4):
            d = 4 * h + dd
            for q in range(O):
                sel[d, h, dd * 20 + q] = q
                sel[d, 2 + h, dd * 20 + q] = ORDER - q

    lc = np.array([np.log(comb(ORDER, q)) for q in range(O)], np.float32)
    logc = np.zeros((80, 2), np.float32)
    for dd in range(4):
        logc[dd * 20 : dd * 20 + 20, 0] = lc
        logc[dd * 20 : dd * 20 + 20, 1] = 2 * lc

    return {
        "wlhs": wlhs.astype(np.float16),
        "w0v": w0v,
        "onesr": onesr.astype(np.float16),
        "sel": sel,
        "logc": logc,
    }, perm


def build_nc(perm, npc=NPC, fd=FD):
    """Emit the bass program (specialized to `perm`, which selects which
    per-dim basis tile gates each pack at each step)."""
    ntiles = npc // fd
    nhalf = fd // 512 if fd >= 512 else 1
    mmfd = min(fd, 512)

    nc = bacc.Bacc(
        "TRN2", target_bir_lowering=False, debug=False, num_devices=NCORES
    )
    Xd = nc.declare_dram_parameter("X", [npc, D], f32, isOutput=False)
    wlhsd = nc.declare_dram_parameter("wlhs", [120, (D - 1) * NPACK * 120], f16, False)
    w0vd = nc.declare_dram_parameter("w0v", [120, NPACK], f32, False)
    onesd = nc.declare_dram_parameter("onesr", [120, NPACK * 2], f16, False)
    seld = nc.declare_dram_parameter("sel", [8, 4 * 80], f32, False)
    logcd = nc.declare_dram_parameter("logc", [80, 2], f32, False)
    Ymd = nc.declare_dram_parameter("Ymean", [npc], f32, isOutput=True)
    Yvd = nc.declare_dram_parameter("Yvar", [npc], f32, isOutput=True)

    with ExitStack() as ctx:
        tc = ctx.enter_context(tile.TileContext(nc))
        wpool = ctx.enter_context(tc.tile_pool(name="w", bufs=1))
        xpool = ctx.enter_context(tc.tile_pool(name="x", bufs=1))
        bpool = ctx.enter_context(tc.tile_pool(name="b", bufs=2))
        vspool = ctx.enter_context(tc.tile_pool(name="vs", bufs=1))
        mspool = ctx.enter_context(tc.tile_pool(name="ms", bufs=2))
        spool = ctx.enter_context(tc.tile_pool(name="st", bufs=1))
        tpool = ctx.enter_context(tc.tile_pool(name="tmp", bufs=3))
        opool = ctx.enter_context(tc.tile_pool(name="oc", bufs=2))
        pmpool = ctx.enter_context(
            tc.tile_pool(name="pm", bufs=2, space="PSUM")
        )
        zpool = ctx.enter_context(tc.tile_pool(name="zh", bufs=1, space="PSUM"))
        rpool = ctx.enter_context(tc.tile_pool(name="red", bufs=1, space="PSUM"))

        # constant loads (once)
        wall = wpool.tile([120, (D - 1) * NPACK, 120], f16)
        nc.sync.dma_start(wall[:], wlhsd.rearrange("r (i c) -> r i c", c=120))
        w0s = wpool.tile([120, NPACK], f32)
        nc.sync.dma_start(w0s[:], w0vd[:])
        oness = wpool.tile([120, NPACK, 2], f16)
        nc.sync.dma_start(oness[:], onesd.rearrange("r (p c) -> r p c", c=2))
        sels = wpool.tile([8, 4, 80], f32)
        nc.sync.dma_start(sels[:], seld.rearrange("r (s c) -> r s c", c=80))
        logcs = wpool.tile([80, 2], f32)
        nc.sync.dma_start(logcs[:], logcd[:])

        for t in range(ntiles):
            n0 = t * fd
            # ---- base tiles: B, B2 per dim (two 80-row halves) ----
            xt = xpool.tile([8, fd], f32, tag="xt")
            nc.sync.dma_start(xt[:], Xd[n0 : n0 + fd, :].rearrange("n d -> d n"))
            nc.vector.tensor_scalar_max(xt[:], xt[:], 1e-30)
            lx = xpool.tile([8, fd], f32, tag="lx")
            l1x = xpool.tile([8, fd], f32, tag="l1x")
            nc.scalar.activation(lx[:], xt[:], AF.Ln)
            nc.scalar.activation(l1x[:], xt[:], AF.Ln, bias=1.0, scale=-1.0)

            bt = []   # B halves [80, fd] f16
            b2t = []  # B^2 halves
            for h in range(2):
                zh = zpool.tile([80, fd], f32, tag="zh")
                for s in range(nhalf):
                    sl = slice(mmfd * s, mmfd * (s + 1))
                    nc.tensor.matmul(
                        zh[:, sl], sels[:, h, :], lx[:, sl], start=True, stop=False
                    )
                    nc.tensor.matmul(
                        zh[:, sl], sels[:, 2 + h, :], l1x[:, sl],
                        start=False, stop=True,
                    )
                bh = bpool.tile([80, fd], f16, tag=f"b{h}")
                b2h = bpool.tile([80, fd], f16, tag=f"b2{h}")
                nc.scalar.activation(bh[:], zh[:], AF.Exp, bias=logcs[:, 0:1])
                nc.scalar.activation(
                    b2h[:], zh[:], AF.Exp, bias=logcs[:, 1:2], scale=2.0
                )
                bt.append(bh)
                b2t.append(b2h)

            def bsrc(d, squared):
                half = b2t[d // 4] if squared else bt[d // 4]
                r0 = (d % 4) * 20
                return half[r0 : r0 + 20, :]

            # ---- var gate stacks: one per dim, 6 replicated blocks ----
            vst = []
            for d in range(D):
                vt = vspool.tile([120, fd], f16, tag=f"vs{d}")
                for c in range(6):
                    nc.sync.dma_start(vt[20 * c : 20 * c + 20, :], bsrc(d, True))
                vst.append(vt)

            def mean_stacks(i):
                out = []
                for g in range(4):
                    mt = mspool.tile([120, fd], f16, tag=f"ms{g}")
                    for j in range(6):
                        mem = GROUPS[g]
                        d = int(perm[mem[j], i]) if j < len(mem) else 0
                        nc.sync.dma_start(mt[20 * j : 20 * j + 20, :], bsrc(d, False))
                    out.append(mt)
                return out

            def gate_stack(pk, i, mst):
                kind, val = PACKS[pk]
                return vst[int(perm[val, i])] if kind == "var" else mst[val]

            # ---- init (step 0): state = stack * w0 (per-partition) ----
            mst = mean_stacks(0)
            state = []
            for pk in range(NPACK):
                st = spool.tile([120, fd], f16, tag=f"st{pk}")
                nc.vector.tensor_scalar_mul(
                    st[:], gate_stack(pk, 0, mst)[:], w0s[:, pk : pk + 1]
                )
                state.append(st)

            # ---- chain steps ----
            for i in range(1, D):
                mst = mean_stacks(i)
                for pk in range(NPACK):
                    wap = wall[:, (i - 1) * NPACK + pk, :]
                    pm = pmpool.tile([120, fd], f32, tag="pm")
                    for s in range(nhalf):
                        sl = slice(mmfd * s, mmfd * (s + 1))
                        nc.tensor.matmul(
                            pm[:, sl], wap, state[pk][:, sl], start=True, stop=True
                        )
                    stk = gate_stack(pk, i, mst)
                    new = spool.tile([120, fd], f16, tag=f"st{pk}")
                    if pk in FUSED_PACKS:
                        nc.vector.tensor_mul(new[:], pm[:], stk[:])
                    else:
                        tmp = tpool.tile([120, fd], f16, tag="tmp")
                        nc.scalar.activation(tmp[:], pm[:], AF.Copy)
                        nc.vector.tensor_mul(new[:], tmp[:], stk[:])
                    state[pk] = new

            # ---- reduce: [mean; var] rows via accumulated ones-matmuls ----
            red = rpool.tile([2, fd], f32, tag="red")
            for s in range(nhalf):
                sl = slice(mmfd * s, mmfd * (s + 1))
                for pk in range(NPACK):
                    nc.tensor.matmul(
                        red[:, sl], oness[:, pk, :], state[pk][:, sl],
                        start=(pk == 0), stop=(pk == NPACK - 1),
                    )
            oc = opool.tile([2, fd], f32, tag="oc")
            nc.vector.tensor_copy(oc[:], red[:])
            nc.sync.dma_start(
                Ymd[n0 : n0 + fd].rearrange("(a n) -> a n", a=1), oc[0:1, :]
            )
            nc.sync.dma_start(
                Yvd[n0 : n0 + fd].rearrange("(a n) -> a n", a=1), oc[1:2, :]
            )

    nc.compile()
    return nc


def kernel(X, perm, meanw0, meanw, varw0, varw):
    consts, perm_np = _prep_consts(perm, meanw0, meanw, varw0, varw)
    nc = build_nc(perm_np)
    X = np.ascontiguousarray(np.asarray(X, np.float32))
    in_maps = []
    for c in range(NCORES):
        m = {"X": X[c * NPC : (c + 1) * NPC]}
        m.update(
            {
                "wlhs": consts["wlhs"].reshape(120, -1),
                "w0v": consts["w0v"],
                "onesr": consts["onesr"].reshape(120, -1),
                "sel": consts["sel"].reshape(8, -1),
                "logc": consts["logc"],
            }
        )
        in_maps.append(m)
    res = run_bass_kernel_spmd(nc, in_maps, list(range(NCORES)))
    outs = []
    for c in range(NCORES):
        r = res.results[c]
        outs.append(np.stack([r["Ymean"], r["Yvar"]], axis=-1))
    return np.concatenate(outs, axis=0).astype(np.float32)



# revision 3
# speedup vs baseline: 1.1485x; 1.1485x over previous
"""Trainium2 Bass kernel for nn_LogBezierButtress.

Math (per point n, per permutation p of the 8 input dims):
  B[d,q]  = C(19,q) x_d^q (1-x_d)^(19-q)          (Bernstein basis, O=20)
  mean chain:  f_0 = exp(meanw0[p]) * B[perm[p,0]]
               f_i = (f_{i-1} @ exp(meanw[i-1,p])) * B[perm[p,i]]
  var chains k=1..6 use weights exp(2*meanw + k*varw) and gate B^2.
  mean(n) = sum_{p,q} f_7 ; var(n) = sum_k c_k sum_{p,q} acc_7[k]

Device mapping (per core, points sharded 8 ways):
  - states live as [120, FD] fp16 SBUF tiles: 6 chains x 20 basis rows,
    points on the free dim. 24 packs: 20 "var" packs (k=1..6 of one p) and
    4 "mean" packs (mean chains of 6 p's).
  - per step: block-diag [120,120] fp16 matmul into PSUM, then gate
    multiply by a basis "stack" tile. Gate stacks are built by SBUF->SBUF
    DMA block copies (split across the Sync and GpSimd queues) from
    per-dim base tiles B / B^2, which are produced on device: Ln(x),
    Ln(1-x) -> selector matmuls (q*lnx+(19-q)*ln1x) -> ACT Exp with
    per-partition log-binomial bias.
  - w0 is folded into the step-1 weights (diag(w0) @ W1), so the chain
    starts directly from the step-0 gate stack; the k-coefficients c_k
    are folded into the last var weights; the final sum over (chains, q)
    is a ones-vector matmul accumulated over packs.
"""

import sys

sys.path.insert(0, "/opt/trn_rl_repo")

from contextlib import ExitStack
from math import comb

import numpy as np

import concourse.bacc as bacc
import concourse.mybir as mybir
import concourse.tile as tile
from concourse.bass_utils import run_bass_kernel_spmd

N, D, ORDER, P = 32768, 8, 19, 20
O = ORDER + 1
NCORES = 8
NPC = N // NCORES  # points per core
FD = 1024          # points per tile (free dim)
NPACK = 24
C_COEF = [1.0, 1 / 2, 1 / 6, 1 / 24, 1 / 120, 1 / 720]
GROUPS = [list(range(g * 6, min(P, g * 6 + 6))) for g in range(4)]
# pack list: ("var", p) x20 then ("mean", group) x4
PACKS = [("var", p) for p in range(P)] + [("mean", g) for g in range(4)]
# packs whose gate runs fused on DVE (PSUM*SBUF->SBUF); the rest use an
# ACT copy (PSUM->SBUF fp16) + DVE fp16 2x multiply. Balance ACT vs DVE.
FUSED_PACKS = set(range(7))

f32 = mybir.dt.float32
f16 = mybir.dt.float16
AF = mybir.ActivationFunctionType


def _prep_consts(perm, meanw0, meanw, varw0, varw):
    """Host-side weight packing (small, O(P*O^2*D))."""
    perm = np.asarray(perm)
    m0 = np.asarray(meanw0, np.float64)
    mw = np.asarray(meanw, np.float64)
    v0 = np.asarray(varw0, np.float64)
    vw = np.asarray(varw, np.float64)

    wlhs = np.zeros((120, D - 1, NPACK, 120), np.float32)
    for i in range(1, D):
        for pk, (kind, val) in enumerate(PACKS):
            for b in range(6):
                if kind == "var":
                    W = np.exp(2 * mw[i - 1, val] + (b + 1) * vw[i - 1, val])
                    if i == 1:
                        w0 = np.exp(2 * m0[val, 0] + (b + 1) * v0[val, 0])
                        W = w0[:, None] * W
                    if i == D - 1:
                        W = W * C_COEF[b]
                else:
                    mem = GROUPS[val]
                    if b < len(mem):
                        W = np.exp(mw[i - 1, mem[b]])
                        if i == 1:
                            W = np.exp(m0[mem[b], 0])[:, None] * W
                    else:
                        W = np.zeros((O, O))
                wlhs[20 * b : 20 * b + 20, i - 1, pk, 20 * b : 20 * b + 20] = W

    onesr = np.zeros((120, NPACK, 2), np.float32)
    for pk, (kind, val) in enumerate(PACKS):
        for b in range(6):
            sl = slice(20 * b, 20 * b + 20)
            if kind == "var":
                onesr[sl, pk, 1] = 1.0
            else:
                mem = GROUPS[val]
                if b < len(mem):
                    onesr[sl, pk, 0] = 1.0

    # selector matmul weights: Z[(d%4)*20+q, n] = q*lnx[d,n] + (19-q)*ln1x[d,n]
    sel = np.zeros((8, 4, 80), np.float32)
    for h in range(2):
        for dd in range(4):
            d = 4 * h + dd
            for q in range(O):
                sel[d, h, dd * 20 + q] = q
                sel[d, 2 + h, dd * 20 + q] = ORDER - q

    lc = np.array([np.log(comb(ORDER, q)) for q in range(O)], np.float32)
    logc = np.zeros((80, 2), np.float32)
    for dd in range(4):
        logc[dd * 20 : dd * 20 + 20, 0] = lc
        logc[dd * 20 : dd * 20 + 20, 1] = 2 * lc

    return {
        "wlhs": wlhs.astype(np.float16),
        "onesr": onesr.astype(np.float16),
        "sel": sel,
        "logc": logc,
    }, perm


def build_nc(perm, npc=NPC, fd=FD):
    """Emit the bass program (specialized to `perm`, which selects which
    per-dim basis tile gates each pack at each step)."""
    ntiles = npc // fd
    nhalf = fd // 512 if fd >= 512 else 1
    mmfd = min(fd, 512)

    nc = bacc.Bacc(
        "TRN2", target_bir_lowering=False, debug=False, num_devices=NCORES
    )
    Xd = nc.declare_dram_parameter("X", [npc, D], f32, isOutput=False)
    wlhsd = nc.declare_dram_parameter("wlhs", [120, (D - 1) * NPACK * 120], f16, False)
    onesd = nc.declare_dram_parameter("onesr", [120, NPACK * 2], f16, False)
    seld = nc.declare_dram_parameter("sel", [8, 4 * 80], f32, False)
    logcd = nc.declare_dram_parameter("logc", [80, 2], f32, False)
    Ymd = nc.declare_dram_parameter("Ymean", [npc], f32, isOutput=True)
    Yvd = nc.declare_dram_parameter("Yvar", [npc], f32, isOutput=True)

    # round-robin the stack-build DMAs over two otherwise-idle queues
    dma_engines = [None, None]

    def stack_dma(dst, src):
        eng = dma_engines[stack_dma.i % 2]
        stack_dma.i += 1
        eng.dma_start(dst, src)

    stack_dma.i = 0

    with ExitStack() as ctx:
        tc = ctx.enter_context(tile.TileContext(nc))
        dma_engines[0] = nc.sync
        dma_engines[1] = nc.gpsimd
        wpool = ctx.enter_context(tc.tile_pool(name="w", bufs=1))
        xpool = ctx.enter_context(tc.tile_pool(name="x", bufs=1))
        bpool = ctx.enter_context(tc.tile_pool(name="b", bufs=2))
        vspool = ctx.enter_context(tc.tile_pool(name="vs", bufs=2))
        mspool = ctx.enter_context(tc.tile_pool(name="ms", bufs=2))
        spool = ctx.enter_context(tc.tile_pool(name="st", bufs=1))
        tpool = ctx.enter_context(tc.tile_pool(name="tmp", bufs=3))
        opool = ctx.enter_context(tc.tile_pool(name="oc", bufs=2))
        pmpool = ctx.enter_context(
            tc.tile_pool(name="pm", bufs=3, space="PSUM")
        )
        zpool = ctx.enter_context(tc.tile_pool(name="zh", bufs=1, space="PSUM"))
        rpool = ctx.enter_context(tc.tile_pool(name="red", bufs=1, space="PSUM"))

        # constant loads (once)
        wall = wpool.tile([120, (D - 1) * NPACK, 120], f16)
        nc.sync.dma_start(wall[:], wlhsd.rearrange("r (i c) -> r i c", c=120))
        oness = wpool.tile([120, NPACK, 2], f16)
        nc.sync.dma_start(oness[:], onesd.rearrange("r (p c) -> r p c", c=2))
        sels = wpool.tile([8, 4, 80], f32)
        nc.sync.dma_start(sels[:], seld.rearrange("r (s c) -> r s c", c=80))
        logcs = wpool.tile([80, 2], f32)
        nc.sync.dma_start(logcs[:], logcd[:])

        for t in range(ntiles):
            n0 = t * fd
            # ---- base tiles: B, B2 per dim (two 80-row halves) ----
            xt = xpool.tile([8, fd], f32, tag="xt")
            nc.sync.dma_start(xt[:], Xd[n0 : n0 + fd, :].rearrange("n d -> d n"))
            nc.vector.tensor_scalar_max(xt[:], xt[:], 1e-30)
            lx = xpool.tile([8, fd], f32, tag="lx")
            l1x = xpool.tile([8, fd], f32, tag="l1x")
            nc.scalar.activation(lx[:], xt[:], AF.Ln)
            nc.scalar.activation(l1x[:], xt[:], AF.Ln, bias=1.0, scale=-1.0)

            bt = []   # B halves [80, fd] f16
            b2t = []  # B^2 halves
            for h in range(2):
                bh = bpool.tile([80, fd], f16, tag=f"b{h}")
                b2h = bpool.tile([80, fd], f16, tag=f"b2{h}")
                for s in range(nhalf):
                    sl = slice(mmfd * s, mmfd * (s + 1))
                    zh = zpool.tile([80, mmfd], f32, tag="zh")
                    nc.tensor.matmul(
                        zh[:], sels[:, h, :], lx[:, sl], start=True, stop=False
                    )
                    nc.tensor.matmul(
                        zh[:], sels[:, 2 + h, :], l1x[:, sl],
                        start=False, stop=True,
                    )
                    nc.scalar.activation(bh[:, sl], zh[:], AF.Exp, bias=logcs[:, 0:1])
                    nc.scalar.activation(
                        b2h[:, sl], zh[:], AF.Exp, bias=logcs[:, 1:2], scale=2.0
                    )
                bt.append(bh)
                b2t.append(b2h)

            def bsrc(d, squared):
                half = b2t[d // 4] if squared else bt[d // 4]
                r0 = (d % 4) * 20
                return half[r0 : r0 + 20, :]

            # ---- var gate stacks: one per dim, 6 replicated blocks ----
            vst = []
            for d in range(D):
                vt = vspool.tile([120, fd], f16, tag=f"vs{d}")
                for c in range(6):
                    stack_dma(vt[20 * c : 20 * c + 20, :], bsrc(d, True))
                vst.append(vt)

            def mean_stacks(i):
                out = []
                for g in range(4):
                    mt = mspool.tile([120, fd], f16, tag=f"ms{g}")
                    for j in range(6):
                        mem = GROUPS[g]
                        d = int(perm[mem[j], i]) if j < len(mem) else 0
                        stack_dma(mt[20 * j : 20 * j + 20, :], bsrc(d, False))
                    out.append(mt)
                return out

            def gate_stack(pk, i, mst):
                kind, val = PACKS[pk]
                return vst[int(perm[val, i])] if kind == "var" else mst[val]

            # ---- chain steps (w0 folded into step 1: rhs is the step-0
            # gate stack itself) ----
            mst = mean_stacks(0)
            state = [None] * NPACK
            for i in range(1, D):
                prev_mst, mst = mst, mean_stacks(i)
                for pk in range(NPACK):
                    rhs = state[pk] if i > 1 else gate_stack(pk, 0, prev_mst)
                    wap = wall[:, (i - 1) * NPACK + pk, :]
                    pm = pmpool.tile([120, fd], f32, tag="pm")
                    for s in range(nhalf):
                        sl = slice(mmfd * s, mmfd * (s + 1))
                        nc.tensor.matmul(
                            pm[:, sl], wap, rhs[:, sl], start=True, stop=True
                        )
                    stk = gate_stack(pk, i, mst)
                    new = spool.tile([120, fd], f16, tag=f"st{pk}")
                    if pk in FUSED_PACKS:
                        nc.vector.tensor_mul(new[:], pm[:], stk[:])
                    else:
                        tmp = tpool.tile([120, fd], f16, tag="tmp")
                        nc.scalar.activation(tmp[:], pm[:], AF.Copy)
                        nc.vector.tensor_mul(new[:], tmp[:], stk[:])
                    state[pk] = new

            # ---- reduce: [mean; var] rows via accumulated ones-matmuls ----
            oc = opool.tile([2, fd], f32, tag="oc")
            for s in range(nhalf):
                sl = slice(mmfd * s, mmfd * (s + 1))
                red = rpool.tile([2, mmfd], f32, tag="red")
                for pk in range(NPACK):
                    nc.tensor.matmul(
                        red[:], oness[:, pk, :], state[pk][:, sl],
                        start=(pk == 0), stop=(pk == NPACK - 1),
                    )
                nc.vector.tensor_copy(oc[:, sl], red[:])
            nc.sync.dma_start(
                Ymd[n0 : n0 + fd].rearrange("(a n) -> a n", a=1), oc[0:1, :]
            )
            nc.sync.dma_start(
                Yvd[n0 : n0 + fd].rearrange("(a n) -> a n", a=1), oc[1:2, :]
            )

    nc.compile()
    return nc


def kernel(X, perm, meanw0, meanw, varw0, varw):
    consts, perm_np = _prep_consts(perm, meanw0, meanw, varw0, varw)
    nc = build_nc(perm_np)
    X = np.ascontiguousarray(np.asarray(X, np.float32))
    in_maps = []
    for c in range(NCORES):
        m = {"X": X[c * NPC : (c + 1) * NPC]}
        m.update(
            {
                "wlhs": consts["wlhs"].reshape(120, -1),
                "onesr": consts["onesr"].reshape(120, -1),
                "sel": consts["sel"].reshape(8, -1),
                "logc": consts["logc"],
            }
        )
        in_maps.append(m)
    res = run_bass_kernel_spmd(nc, in_maps, list(range(NCORES)))
    outs = []
    for c in range(NCORES):
        r = res.results[c]
        outs.append(np.stack([r["Ymean"], r["Yvar"]], axis=-1))
    return np.concatenate(outs, axis=0).astype(np.float32)
